# revision 7
# baseline (speedup 1.0000x reference)
"""Transformer-XL CompressiveLayer on 8 TRN2 NeuronCores.

Sharding: core c = (batch b = c//2) x (head-half hh = c%2).
Each core handles one batch's full 896 tokens with 8 of 16 heads and
2048 of 4096 FFN inner channels.  One pairwise AllReduce joins the
attention output halves before the post-LN; the FFN partial outputs are
summed on the host (plus ff2 bias).

All matmuls run in bf16 with fp32 PSUM accumulation; LayerNorm, softmax
(exp/sum/recip) run in fp32.  rel_shift is realized exactly via a padded
DRAM buffer: BD^T tiles are written at flat offset 897*i + 1 + j and the
shifted matrix is read back at flat offset 896 + 896*i + j.
"""

import numpy as np
import ml_dtypes
from contextlib import ExitStack

import concourse.bass as bass
import concourse.tile as tile
from concourse import mybir, bacc
from concourse.bass_utils import run_bass_kernel_spmd
from concourse.masks import make_identity

F32 = mybir.dt.float32
BF16 = mybir.dt.bfloat16
BF = ml_dtypes.bfloat16

D, H, DH, FF = 1024, 16, 64, 4096
S, B, M, CM = 512, 4, 256, 128
K = S + M + CM          # 896 tokens
NT = K // 128           # 7 token tiles
ND = D // 128           # 8 D chunks
HC = 8                  # heads per core
FC = HC * DH            # 512 features per core
NFC = FC // 128         # 4 feature chunks per core
FFC = FF // 2           # 2048 ffn channels per core
NFF = FFC // 128        # 16 ffn chunks per core
SCALE = 1.0 / np.sqrt(DH)
EPS = 1e-5
# i-halves of the 896 columns: [0:512] and [512:896]
HALVES = [(0, 512), (512, 384)]

_CACHED = {}


def _ln_tile(nc, pool, x_t, eps_sb, stat_pool):
    """In: x_t [128,1024] f32. Returns (mean, rstd) [128,1] tiles."""
    stats = stat_pool.tile([128, 2, 6], F32, tag="stats")
    for g in range(2):
        nc.vector.bn_stats(out=stats[:, g, :], in_=x_t[:, g * 512:(g + 1) * 512])
    mv = stat_pool.tile([128, 2], F32, tag="mv")
    nc.vector.bn_aggr(out=mv, in_=stats)
    rstd = stat_pool.tile([128, 1], F32, tag="rstd")
    nc.scalar.activation(out=rstd, in_=mv[:, 1:2],
                         func=mybir.ActivationFunctionType.Sqrt,
                         bias=eps_sb, scale=1.0)
    nc.vector.reciprocal(out=rstd, in_=rstd)
    return mv[:, 0:1], rstd


def build(debug=False, sim_mode=False):
    nc = bacc.Bacc(None)

    xb = nc.declare_dram_parameter("xb", [K, D], F32, isOutput=False)
    posT = nc.declare_dram_parameter("posT", [128, ND, K], BF16, isOutput=False)
    qw = nc.declare_dram_parameter("qw", [128, ND, FC], BF16, isOutput=False)
    kw = nc.declare_dram_parameter("kw", [128, ND, FC], BF16, isOutput=False)
    vw = nc.declare_dram_parameter("vw", [128, ND, FC], BF16, isOutput=False)
    rw = nc.declare_dram_parameter("rw", [128, ND, FC], BF16, isOutput=False)
    ow = nc.declare_dram_parameter("ow", [128, NFC, D], BF16, isOutput=False)
    ff1w = nc.declare_dram_parameter("ff1w", [128, ND, FFC], BF16, isOutput=False)
    ff2w = nc.declare_dram_parameter("ff2w", [128, NFF, D], BF16, isOutput=False)
    rwb = nc.declare_dram_parameter("rwb", [128, NFC], F32, isOutput=False)
    rrb = nc.declare_dram_parameter("rrb", [128, NFC], F32, isOutput=False)
    ff1b = nc.declare_dram_parameter("ff1b", [128, NFF], F32, isOutput=False)
    ln1w = nc.declare_dram_parameter("ln1w", [D], F32, isOutput=False)
    ln1b = nc.declare_dram_parameter("ln1b", [D], F32, isOutput=False)
    ln2w = nc.declare_dram_parameter("ln2w", [D], F32, isOutput=False)
    ln2b = nc.declare_dram_parameter("ln2b", [D], F32, isOutput=False)

    out = nc.declare_dram_parameter("out", [K, D], F32, isOutput=True)
    if debug:
        dbg_w = nc.declare_dram_parameter("dbg_w", [K, D], F32, isOutput=True)
        dbg_pad = nc.declare_dram_parameter("dbg_pad", [K * (K + 1)], BF16,
                                            isOutput=True)
        dbg_ao = nc.declare_dram_parameter("dbg_ao", [K, D], F32, isOutput=True)
        dbg_ares = nc.declare_dram_parameter("dbg_ares", [K, D], F32,
                                             isOutput=True)

    w_dram = nc.dram_tensor("w_dram", [K, D], F32)
    pads = [nc.dram_tensor(f"pad{i}", [K * (K + 1)], BF16) for i in range(2)]
    ao_in = nc.dram_tensor("ao_in", [K, D], F32)
    ao_out = nc.dram_tensor("ao_out", [K, D], F32)

    with tile.TileContext(nc) as tc, ExitStack() as ctx:
        consts = ctx.enter_context(tc.tile_pool(name="consts", bufs=1))
        psS = ctx.enter_context(tc.tile_pool(name="psS", bufs=6, space="PSUM"))
        psTr = ctx.enter_context(tc.tile_pool(name="psTr", bufs=2, space="PSUM"))
        stat_pool = ctx.enter_context(tc.tile_pool(name="stats", bufs=4))

        ident = consts.tile([128, 128], BF16)
        make_identity(nc, ident)
        eps_sb = consts.tile([128, 1], F32)
        nc.vector.memset(eps_sb, EPS)
        zrow = consts.tile([128, NT], BF16)
        nc.vector.memset(zrow, 0.0)
        rwb_sb = consts.tile([128, NFC], F32)
        nc.sync.dma_start(out=rwb_sb, in_=rwb[:])
        rrb_sb = consts.tile([128, NFC], F32)
        nc.sync.dma_start(out=rrb_sb, in_=rrb[:])
        ff1b_sb = consts.tile([128, NFF], F32)
        nc.sync.dma_start(out=ff1b_sb, in_=ff1b[:])

        def bcast(name, param):
            t = consts.tile([128, D], F32, tag=name)
            nc.sync.dma_start(out=t, in_=bass.AP(tensor=param, offset=0,
                                                 ap=[[0, 128], [1, D]]))
            return t

        ln1w_bc = bcast("ln1w_bc", ln1w)
        ln1b_bc = bcast("ln1b_bc", ln1b)
        ln2w_bc = bcast("ln2w_bc", ln2w)
        ln2b_bc = bcast("ln2b_bc", ln2b)

        # zero column 0 of both pad buffers (written once; BD writes never
        # touch column 0)
        for pad in pads:
            nc.sync.dma_start(
                out=bass.AP(tensor=pad, offset=0,
                            ap=[[K + 1, 128], [(K + 1) * 128, NT]]),
                in_=zrow)

        avT = ctx.enter_context(tc.tile_pool(name="avTp", bufs=1))
        avT_sb = avT.tile([128, NFC, K], BF16)

        attn = ctx.enter_context(tc.tile_pool(name="attn", bufs=1))
        q1T = attn.tile([128, NFC, K], BF16)
        q2T = attn.tile([128, NFC, K], BF16)
        kT = attn.tile([128, NFC, K], BF16)
        rT = attn.tile([128, NFC, K], BF16)
        v1 = attn.tile([128, NT, HC, DH + 1], BF16)
        nc.vector.memset(v1, 1.0)

        # ---------------- P0: LN1, transpose, projections ----------------
        with tc.tile_pool(name="p0", bufs=3) as p0, \
             tc.tile_pool(name="p0w", bufs=1) as p0w:
            posT_sb = p0w.tile([128, ND, K], BF16)
            nc.sync.dma_start(out=posT_sb, in_=posT[:])
            qw_sb = p0w.tile([128, ND, FC], BF16)
            nc.sync.dma_start(out=qw_sb, in_=qw[:])
            kw_sb = p0w.tile([128, ND, FC], BF16)
            nc.sync.dma_start(out=kw_sb, in_=kw[:])
            vw_sb = p0w.tile([128, ND, FC], BF16)
            nc.sync.dma_start(out=vw_sb, in_=vw[:])
            rw_sb = p0w.tile([128, ND, FC], BF16)
            nc.sync.dma_start(out=rw_sb, in_=rw[:])
            wT = p0w.tile([128, ND, K], BF16)

            for t in range(NT):
                x_t = p0.tile([128, D], F32, tag="x")
                nc.sync.dma_start(out=x_t, in_=xb[t * 128:(t + 1) * 128, :])
                mean, rstd = _ln_tile(nc, p0, x_t, eps_sb, stat_pool)
                wn = p0.tile([128, D], F32, tag="wn")
                nc.vector.tensor_scalar(out=wn, in0=x_t, scalar1=mean,
                                        scalar2=rstd,
                                        op0=mybir.AluOpType.subtract,
                                        op1=mybir.AluOpType.mult)
                wg = p0.tile([128, D], F32, tag="wg")
                nc.gpsimd.tensor_mul(out=wg, in0=wn, in1=ln1w_bc)
                wf = p0.tile([128, D], F32, tag="wf")
                nc.gpsimd.tensor_add(out=wf, in0=wg, in1=ln1b_bc)
                nc.sync.dma_start(out=w_dram[t * 128:(t + 1) * 128, :], in_=wf)
                wb = p0.tile([128, D], BF16, tag="wb")
                nc.scalar.copy(out=wb, in_=wf)
                for dc in range(ND):
                    ps = psTr.tile([128, 128], BF16, tag="tr")
                    nc.tensor.transpose(ps, wb[:, dc * 128:(dc + 1) * 128],
                                        ident)
                    nc.scalar.copy(out=wT[:, dc, t * 128:(t + 1) * 128],
                                   in_=ps)

            # q^T (with biases), k^T, r^T : [feat, tok]
            for fc in range(NFC):
                for (i0, n) in HALVES:
                    pq = psS.tile([128, 512], F32, tag="s")
                    pk = psS.tile([128, 512], F32, tag="s")
                    pr = psS.tile([128, 512], F32, tag="s")
                    for dc in range(ND):
                        st = dict(start=(dc == 0), stop=(dc == ND - 1))
                        lq = qw_sb[:, dc, fc * 128:(fc + 1) * 128]
                        lk = kw_sb[:, dc, fc * 128:(fc + 1) * 128]
                        lr = rw_sb[:, dc, fc * 128:(fc + 1) * 128]
                        nc.tensor.matmul(pq[:, :n], lq,
                                         wT[:, dc, i0:i0 + n], **st)
                        nc.tensor.matmul(pk[:, :n], lk,
                                         wT[:, dc, i0:i0 + n], **st)
                        nc.tensor.matmul(pr[:, :n], lr,
                                         posT_sb[:, dc, i0:i0 + n], **st)
                    nc.vector.tensor_scalar_add(out=q1T[:, fc, i0:i0 + n],
                                                in0=pq[:, :n],
                                                scalar1=rwb_sb[:, fc:fc + 1])
                    nc.vector.tensor_scalar_add(out=q2T[:, fc, i0:i0 + n],
                                                in0=pq[:, :n],
                                                scalar1=rrb_sb[:, fc:fc + 1])
                    nc.scalar.copy(out=kT[:, fc, i0:i0 + n], in_=pk[:, :n])
                    nc.scalar.copy(out=rT[:, fc, i0:i0 + n], in_=pr[:, :n])

            # v : [tok, feat] with ones column per head
            for t in range(NT):
                pv = psS.tile([128, 512], F32, tag="s")
                for dc in range(ND):
                    nc.tensor.matmul(pv, wT[:, dc, t * 128:(t + 1) * 128],
                                     vw_sb[:, dc, :],
                                     start=(dc == 0), stop=(dc == ND - 1))
                nc.vector.tensor_copy(
                    out=v1[:, t, :, 0:DH],
                    in_=pv.rearrange("p (h d) -> p h d", h=HC))

        if debug:
            for t in range(NT):
                tmp = stat_pool.tile([128, D], F32, tag="dbgw")
                nc.sync.dma_start(out=tmp, in_=w_dram[t * 128:(t + 1) * 128, :])
                nc.sync.dma_start(out=dbg_w[t * 128:(t + 1) * 128, :], in_=tmp)

        # ---------------- P1: per-head attention ----------------
        with tc.tile_pool(name="p1", bufs=4) as p1, \
             tc.tile_pool(name="pt", bufs=18) as ptp, \
             tc.tile_pool(name="p1s", bufs=2) as p1s:
            av2 = p1s.tile([128, NT, NFC, 128], BF16, tag="av2")
            for h in range(HC):
                pad = pads[h % 2]
                hp = (h % 2) * DH
                fc = h // 2
                # BD^T -> pad buffer
                for jt in range(NT):
                    for hi, (i0, n) in enumerate(HALVES):
                        pb = psS.tile([128, 512], F32, tag="s")
                        nc.tensor.matmul(
                            pb[:, :n],
                            rT[hp:hp + DH, fc, jt * 128:(jt + 1) * 128],
                            q2T[hp:hp + DH, fc, i0:i0 + n],
                            start=True, stop=True)
                        bd = p1.tile([128, 512], BF16, tag="bd")
                        nc.scalar.copy(out=bd[:, :n], in_=pb[:, :n])
                        nc.sync.dma_start(
                            out=bass.AP(tensor=pad,
                                        offset=(K + 1) * i0 + 1 + 128 * jt,
                                        ap=[[1, 128], [K + 1, n]]),
                            in_=bd[:, :n])
                if debug and h == 0:
                    # K*(K+1) = 448 * 1794; dump in [rows,448] strips
                    total_rows = K * (K + 1) // 448
                    for r0 in range(0, total_rows, 128):
                        rows = min(128, total_rows - r0)
                        tmp = p1.tile([128, 512], BF16, tag="bd")
                        nc.sync.dma_start(
                            out=tmp[:rows, :448],
                            in_=bass.AP(tensor=pad, offset=r0 * 448,
                                        ap=[[448, rows], [1, 448]]))
                        nc.sync.dma_start(
                            out=bass.AP(tensor=dbg_pad, offset=r0 * 448,
                                        ap=[[448, rows], [1, 448]]),
                            in_=tmp[:rows, :448])

                # S^T = AC^T + shift(BD^T) ; P = exp(S*scale) ; PV
                pts = {}
                for jt in range(NT):
                    for hi, (i0, n) in enumerate(HALVES):
                        sbd = p1.tile([128, 512], BF16, tag="sbd")
                        nc.sync.dma_start(
                            out=sbd[:, :n],
                            in_=bass.AP(tensor=pad,
                                        offset=K + K * i0 + 128 * jt,
                                        ap=[[1, 128], [K, n]]))
                        ps = psS.tile([128, 512], F32, tag="s")
                        nc.tensor.matmul(
                            ps[:, :n],
                            kT[hp:hp + DH, fc, jt * 128:(jt + 1) * 128],
                            q1T[hp:hp + DH, fc, i0:i0 + n],
                            start=True, stop=True)
                        nc.vector.tensor_add(out=ps[:, :n], in0=ps[:, :n],
                                             in1=sbd[:, :n])
                        pt = ptp.tile([128, 512], BF16, tag="pt")
                        nc.scalar.activation(
                            out=pt[:, :n], in_=ps[:, :n],
                            func=mybir.ActivationFunctionType.Exp,
                            scale=float(SCALE))
                        pts[(jt, hi)] = pt
                for it in range(NT):
                    hi = 0 if it < 4 else 1
                    il = it * 128 - hi * 512
                    pv = psTr.tile([128, DH + 1], F32, tag="tr")
                    for jt in range(NT):
                        nc.tensor.matmul(pv, pts[(jt, hi)][:, il:il + 128],
                                         v1[:, jt, h, :],
                                         start=(jt == 0), stop=(jt == NT - 1))
                    rcp = stat_pool.tile([128, 1], F32, tag="rcp")
                    nc.vector.reciprocal(out=rcp, in_=pv[:, DH:DH + 1])
                    nc.vector.tensor_scalar_mul(
                        out=av2[:, it, fc, hp:hp + DH],
                        in0=pv[:, 0:DH], scalar1=rcp)

            # transpose attn_vec -> [feat, tok]
            for it in range(NT):
                for p in range(NFC):
                    ps = psTr.tile([128, 128], BF16, tag="tr")
                    nc.tensor.transpose(ps, av2[:, it, p, :], ident)
                    nc.vector.tensor_copy(
                        out=avT_sb[:, p, it * 128:(it + 1) * 128], in_=ps)

        # ---------------- P2: o_proj, AllReduce, LN2, transpose ----------
        arT = ctx.enter_context(tc.tile_pool(name="arTp", bufs=1))
        arT_sb = arT.tile([128, ND, K], BF16)

        with tc.tile_pool(name="p2", bufs=3) as p2, \
             tc.tile_pool(name="p2w", bufs=1) as p2w:
            ow_sb = p2w.tile([128, NFC, D], BF16)
            nc.sync.dma_start(out=ow_sb, in_=ow[:])
            for it in range(NT):
                po = [psS.tile([128, 512], F32, tag="s", name=f"po{i}")
                      for i in range(2)]
                for fc in range(NFC):
                    st = dict(start=(fc == 0), stop=(fc == NFC - 1))
                    l = avT_sb[:, fc, it * 128:(it + 1) * 128]
                    nc.tensor.matmul(po[0], l, ow_sb[:, fc, 0:512], **st)
                    nc.tensor.matmul(po[1], l, ow_sb[:, fc, 512:1024], **st)
                ao = p2.tile([128, D], F32, tag="ao")
                nc.vector.tensor_copy(out=ao[:, 0:512], in_=po[0])
                nc.vector.tensor_copy(out=ao[:, 512:1024], in_=po[1])
                nc.sync.dma_start(out=ao_in[it * 128:(it + 1) * 128, :], in_=ao)

            if sim_mode:
                # TimelineSim can't model collectives; substitute a DRAM copy
                # of comparable size for profiling runs.
                nc.sync.dma_start(out=ao_out[:], in_=ao_in[:])
            else:
                nc.gpsimd.collective_compute(
                    "AllReduce", mybir.AluOpType.add,
                    replica_groups=[[0, 1], [2, 3], [4, 5], [6, 7]],
                    ins=[ao_in[:]], outs=[ao_out[:]])

            for it in range(NT):
                aor = p2.tile([128, D], F32, tag="aor")
                nc.sync.dma_start(out=aor,
                                  in_=ao_out[it * 128:(it + 1) * 128, :])
                if debug:
                    nc.sync.dma_start(out=dbg_ao[it * 128:(it + 1) * 128, :],
                                      in_=aor)
                wr = p2.tile([128, D], F32, tag="wr")
                nc.sync.dma_start(out=wr,
                                  in_=w_dram[it * 128:(it + 1) * 128, :])
                x2 = p2.tile([128, D], F32, tag="x2")
                nc.vector.tensor_add(out=x2, in0=aor, in1=wr)
                mean, rstd = _ln_tile(nc, p2, x2, eps_sb, stat_pool)
                xn = p2.tile([128, D], F32, tag="xn")
                nc.vector.tensor_scalar(out=xn, in0=x2, scalar1=mean,
                                        scalar2=rstd,
                                        op0=mybir.AluOpType.subtract,
                                        op1=mybir.AluOpType.mult)
                xg = p2.tile([128, D], F32, tag="xg")
                nc.gpsimd.tensor_mul(out=xg, in0=xn, in1=ln2w_bc)
                ars = p2.tile([128, D], BF16, tag="ars")
                nc.vector.tensor_add(out=ars, in0=xg, in1=ln2b_bc)
                if debug:
                    arf = p2.tile([128, D], F32, tag="arf")
                    nc.scalar.copy(out=arf, in_=ars)
                    nc.sync.dma_start(out=dbg_ares[it * 128:(it + 1) * 128, :],
                                      in_=arf)
                for dc in range(ND):
                    ps = psTr.tile([128, 128], BF16, tag="tr")
                    nc.tensor.transpose(ps, ars[:, dc * 128:(dc + 1) * 128],
                                        ident)
                    nc.scalar.copy(out=arT_sb[:, dc, it * 128:(it + 1) * 128],
                                   in_=ps)

        # ---------------- P3: FFN ----------------
        with tc.tile_pool(name="p3", bufs=3) as p3, \
             tc.tile_pool(name="ff1s", bufs=6) as ff1s, \
             tc.tile_pool(name="p3w", bufs=1) as p3w:
            ff2w_sb = p3w.tile([128, NFF, D], BF16)
            nc.sync.dma_start(out=ff2w_sb, in_=ff2w[:])
            hT = p3w.tile([128, NFF, K], BF16)
            for ffc in range(NFF):
                for (i0, n) in HALVES:
                    ph = psS.tile([128, 512], F32, tag="s")
                    for dc in range(ND):
                        wt = ff1s.tile([128, 128], BF16, tag="ff1t")
                        nc.sync.dma_start(
                            out=wt,
                            in_=ff1w[:, dc, ffc * 128:(ffc + 1) * 128])
                        nc.tensor.matmul(ph[:, :n], wt, arT_sb[:, dc, i0:i0 + n],
                                         start=(dc == 0), stop=(dc == ND - 1))
                    nc.scalar.activation(
                        out=hT[:, ffc, i0:i0 + n], in_=ph[:, :n],
                        func=mybir.ActivationFunctionType.Relu,
                        bias=ff1b_sb[:, ffc:ffc + 1], scale=1.0)
            for it in range(NT):
                po = [psS.tile([128, 512], F32, tag="s", name=f"pf{i}")
                      for i in range(2)]
                for ffc in range(NFF):
                    st = dict(start=(ffc == 0), stop=(ffc == NFF - 1))
                    l = hT[:, ffc, it * 128:(it + 1) * 128]
                    nc.tensor.matmul(po[0], l, ff2w_sb[:, ffc, 0:512], **st)
                    nc.tensor.matmul(po[1], l, ff2w_sb[:, ffc, 512:1024], **st)
                ot = p3.tile([128, D], F32, tag="ot")
                nc.vector.tensor_copy(out=ot[:, 0:512], in_=po[0])
                nc.vector.tensor_copy(out=ot[:, 512:1024], in_=po[1])
                nc.sync.dma_start(out=out[it * 128:(it + 1) * 128, :], in_=ot)

    nc.finalize()
    return nc


def prep_inputs(inputs):
    """Full inputs -> list of 8 per-core input maps."""
    x_nat = np.concatenate([inputs["input_ids"], inputs["mem"],
                            inputs["c_mem"]], axis=0)  # [K,B,D] f32
    posT_full = np.ascontiguousarray(inputs["positional_embedding"].T) \
        .astype(BF)  # [D,K]
    posT_t = posT_full.reshape(ND, 128, K).transpose(1, 0, 2).copy()

    qkv = inputs["qkv_w"]
    maps = []
    for c in range(8):
        b, hh = c // 2, c % 2
        Fs = slice(hh * FC, (hh + 1) * FC)
        FFs = slice(hh * FFC, (hh + 1) * FFC)

        def wchunk(wmat):  # [D, FC] -> [128, ND, FC] bf16
            return np.ascontiguousarray(
                wmat.astype(BF).reshape(ND, 128, -1).transpose(1, 0, 2))

        m = {
            "xb": np.ascontiguousarray(x_nat[:, b, :], np.float32),
            "posT": posT_t,
            "qw": wchunk(qkv[:, 0 * H * DH:1 * H * DH][:, Fs]),
            "kw": wchunk(qkv[:, 1 * H * DH:2 * H * DH][:, Fs]),
            "vw": wchunk(qkv[:, 2 * H * DH:3 * H * DH][:, Fs]),
            "rw": wchunk(inputs["r_w"][:, Fs]),
            "ow": np.ascontiguousarray(
                inputs["o_w"][Fs, :].astype(BF)
                .reshape(NFC, 128, D).transpose(1, 0, 2)),
            "ff1w": wchunk(inputs["ff1_w"][:, FFs]),
            "ff2w": np.ascontiguousarray(
                inputs["ff2_w"][FFs, :].astype(BF)
                .reshape(NFF, 128, D).transpose(1, 0, 2)),
            "rwb": np.ascontiguousarray(
                inputs["r_w_bias"][hh * HC:(hh + 1) * HC].reshape(-1)
                .reshape(NFC, 128).T.astype(np.float32)),
            "rrb": np.ascontiguousarray(
                inputs["r_r_bias"][hh * HC:(hh + 1) * HC].reshape(-1)
                .reshape(NFC, 128).T.astype(np.float32)),
            "ff1b": np.ascontiguousarray(
                inputs["ff1_b"][FFs].reshape(NFF, 128).T.astype(np.float32)),
            "ln1w": np.asarray(inputs["ln1_w"], np.float32),
            "ln1b": np.asarray(inputs["ln1_b"], np.float32),
            "ln2w": np.asarray(inputs["ln2_w"], np.float32),
            "ln2b": np.asarray(inputs["ln2_b"], np.float32),
        }
        maps.append(m)
    return maps


class PjrtRunner:
    """Persistent jitted SPMD executor for a prebuilt Bass module.

    Mirrors bass2jax.run_bass_via_pjrt but keeps the jitted callable so
    repeated invocations (for steady-state timing) skip retrace/recompile.
    Donation is disabled; every output element is written by the kernel.
    """

    def __init__(self, nc, n_cores=8):
        import jax
        from jax.sharding import Mesh, PartitionSpec
        from jax.experimental.shard_map import shard_map
        from concourse import mybir as _mybir
        from concourse.bass2jax import (_bass_exec_p, install_neuronx_cc_hook,
                                        partition_id_tensor)
        install_neuronx_cc_hook()
        self.jax = jax
        self.n_cores = n_cores
        in_names, out_names, out_avals = [], [], []
        partition_name = (nc.partition_id_tensor.name
                          if nc.partition_id_tensor else None)
        for alloc in nc.m.functions[0].allocations:
            if not isinstance(alloc, _mybir.MemoryLocationSet):
                continue
            name = alloc.memorylocations[0].name
            if alloc.kind == "ExternalInput":
                if name != partition_name:
                    in_names.append(name)
            elif alloc.kind == "ExternalOutput":
                out_names.append(name)
                out_avals.append(jax.core.ShapedArray(
                    tuple(alloc.tensor_shape), _mybir.dt.np(alloc.dtype)))
        self.in_names, self.out_names, self.out_avals = \
            in_names, out_names, out_avals

        def _body(*args):
            operands = list(args)
            if partition_name is not None:
                operands.append(partition_id_tensor())
            all_in = in_names + out_names
            if partition_name is not None:
                all_in = all_in + [partition_name]
            return tuple(_bass_exec_p.bind(
                *operands,
                out_avals=tuple(out_avals),
                in_names=tuple(all_in),
                out_names=tuple(out_names),
                lowering_input_output_aliases=(),
                sim_require_finite=True,
                sim_require_nnan=True,
                nc=nc,
            ))

        devices = jax.devices()[:n_cores]
        self.mesh = Mesh(np.asarray(devices), ("core",))
        nin = len(in_names) + len(out_names)
        self.fn = jax.jit(shard_map(
            _body, mesh=self.mesh,
            in_specs=(PartitionSpec("core"),) * nin,
            out_specs=(PartitionSpec("core"),) * len(out_names),
            check_rep=False))

    def pack(self, maps):
        """Per-core input maps -> list of concatenated device arrays."""
        arrs = [self.jax.device_put(
                    np.concatenate([np.asarray(maps[c][n])
                                    for c in range(self.n_cores)], axis=0))
                for n in self.in_names]
        arrs += [self.jax.device_put(
                    np.zeros((self.n_cores * a.shape[0], *a.shape[1:]),
                             a.dtype))
                 for a in self.out_avals]
        return arrs

    def __call__(self, packed):
        return self.fn(*packed)

    def unpack(self, outs):
        res = []
        for c in range(self.n_cores):
            res.append({
                n: np.asarray(outs[i]).reshape(
                    self.n_cores, *self.out_avals[i].shape)[c]
                for i, n in enumerate(self.out_names)})
        return res


def get_runner(debug=False, sim_mode=False):
    key = (bool(debug), bool(sim_mode))
    if key not in _CACHED:
        nc = build(debug=debug, sim_mode=sim_mode)
        _CACHED[key] = PjrtRunner(nc, 8)
    return _CACHED[key]


def _assemble(inputs, results):
    ff2b = np.asarray(inputs["ff2_b"], np.float32)
    out = np.zeros((K, B, D), np.float32)
    for b in range(B):
        out[:, b, :] = (np.asarray(results[2 * b]["out"], np.float32)
                        + np.asarray(results[2 * b + 1]["out"], np.float32)
                        + ff2b[None, :])
    return out


def run(inputs, trace=False, debug=False):
    runner = get_runner(debug=debug)
    maps = prep_inputs(inputs)
    packed = runner.pack(maps)
    outs = runner(packed)
    results = runner.unpack(outs)

    class R:
        pass
    res = R()
    res.results = results
    res.exec_time_ns = None
    return _assemble(inputs, results), res


def kernel(**inputs):
    inputs = {k: np.asarray(v) for k, v in inputs.items()}
    out, _ = run(inputs, trace=False, debug=False)
    return out


# revision 17
# speedup vs baseline: 2.0174x; 2.0174x over previous
"""Transformer-XL CompressiveLayer on 8 TRN2 NeuronCores.

Sharding: core c = (batch b = c//2) x (head-half hh = c%2).
Each core handles one batch's full 896 tokens with 8 of 16 heads and
2048 of 4096 FFN inner channels.  One pairwise AllReduce joins the
attention output halves before the post-LN; the FFN partial outputs are
summed on the host (plus ff2 bias).

All matmuls run in bf16 with fp32 PSUM accumulation; LayerNorm, softmax
(exp/sum/recip) run in fp32.  rel_shift is realized exactly via a padded
DRAM buffer: BD^T tiles are written at flat offset 897*i + 1 + j and the
shifted matrix is read back at flat offset 896 + 896*i + j.
"""

import numpy as np
import ml_dtypes
from contextlib import ExitStack

import concourse.bass as bass
import concourse.tile as tile
from concourse import mybir, bacc
from concourse.bass_utils import run_bass_kernel_spmd
from concourse.masks import make_identity

F32 = mybir.dt.float32
BF16 = mybir.dt.bfloat16
BF = ml_dtypes.bfloat16

D, H, DH, FF = 1024, 16, 64, 4096
S, B, M, CM = 512, 4, 256, 128
K = S + M + CM          # 896 tokens
NT = K // 128           # 7 token tiles
ND = D // 128           # 8 D chunks
HC = 8                  # heads per core
FC = HC * DH            # 512 features per core
NFC = FC // 128         # 4 feature chunks per core
FFC = FF // 2           # 2048 ffn channels per core
NFF = FFC // 128        # 16 ffn chunks per core
SCALE = 1.0 / np.sqrt(DH)
EPS = 1e-5
# i-halves of the 896 columns: [0:512] and [512:896]
HALVES = [(0, 512), (512, 384)]

_CACHED = {}


def _ln_tile(nc, pool, x_t, eps_sb, stat_pool):
    """In: x_t [128,1024] f32. Returns (mean, rstd) [128,1] tiles."""
    stats = stat_pool.tile([128, 2, 6], F32, tag="stats")
    for g in range(2):
        nc.vector.bn_stats(out=stats[:, g, :], in_=x_t[:, g * 512:(g + 1) * 512])
    mv = stat_pool.tile([128, 2], F32, tag="mv")
    nc.vector.bn_aggr(out=mv, in_=stats)
    rstd = stat_pool.tile([128, 1], F32, tag="rstd")
    nc.scalar.activation(out=rstd, in_=mv[:, 1:2],
                         func=mybir.ActivationFunctionType.Sqrt,
                         bias=eps_sb, scale=1.0)
    nc.vector.reciprocal(out=rstd, in_=rstd)
    return mv[:, 0:1], rstd


def build(debug=False, sim_mode=False):
    nc = bacc.Bacc(None)

    xb = nc.declare_dram_parameter("xb", [K, D], F32, isOutput=False)
    posT = nc.declare_dram_parameter("posT", [128, NT, ND, 128], BF16,
                                     isOutput=False)
    qw = nc.declare_dram_parameter("qw", [128, ND, FC], BF16, isOutput=False)
    kw = nc.declare_dram_parameter("kw", [128, ND, FC], BF16, isOutput=False)
    vw = nc.declare_dram_parameter("vw", [128, ND, FC], BF16, isOutput=False)
    rw = nc.declare_dram_parameter("rw", [128, ND, FC], BF16, isOutput=False)
    ow = nc.declare_dram_parameter("ow", [128, NFC, D], BF16, isOutput=False)
    ff1w = nc.declare_dram_parameter("ff1w", [128, ND, FFC], BF16, isOutput=False)
    ff2w = nc.declare_dram_parameter("ff2w", [128, NFF, D], BF16, isOutput=False)
    rwb = nc.declare_dram_parameter("rwb", [128, NFC], F32, isOutput=False)
    rrb = nc.declare_dram_parameter("rrb", [128, NFC], F32, isOutput=False)
    ff1b = nc.declare_dram_parameter("ff1b", [128, NFF], F32, isOutput=False)
    ln1w = nc.declare_dram_parameter("ln1w", [D], F32, isOutput=False)
    ln1b = nc.declare_dram_parameter("ln1b", [D], F32, isOutput=False)
    ln2w = nc.declare_dram_parameter("ln2w", [D], F32, isOutput=False)
    ln2b = nc.declare_dram_parameter("ln2b", [D], F32, isOutput=False)

    out = nc.declare_dram_parameter("out", [K, D], F32, isOutput=True)
    if debug:
        dbg_w = nc.declare_dram_parameter("dbg_w", [K, D], F32, isOutput=True)
        dbg_pad = nc.declare_dram_parameter("dbg_pad", [K * (K + 1)], BF16,
                                            isOutput=True)
        dbg_ao = nc.declare_dram_parameter("dbg_ao", [K, D], F32, isOutput=True)
        dbg_ares = nc.declare_dram_parameter("dbg_ares", [K, D], F32,
                                             isOutput=True)

    w_dram = nc.dram_tensor("w_dram", [K, D], F32)
    pads = [nc.dram_tensor(f"pad{i}", [K * (K + 1)], BF16) for i in range(2)]
    ao_in = nc.dram_tensor("ao_in", [K, D], F32)
    ao_out = nc.dram_tensor("ao_out", [K, D], F32)

    with tile.TileContext(nc) as tc, ExitStack() as ctx:
        consts = ctx.enter_context(tc.tile_pool(name="consts", bufs=1))
        # One PSUM pool: tag "s" = [128,1024] f32 (2 banks) x3, tag "pv" =
        # [64,512] f32 (1 bank) x2 -> 8 banks total.
        psS = ctx.enter_context(tc.tile_pool(name="psS", bufs=3, space="PSUM"))
        stat_pool = ctx.enter_context(tc.tile_pool(name="stats", bufs=4))

        eps_sb = consts.tile([128, 1], F32)
        nc.vector.memset(eps_sb, EPS)
        zrow = consts.tile([128, NT], BF16)
        nc.vector.memset(zrow, 0.0)
        rwb_sb = consts.tile([128, NFC], F32)
        nc.sync.dma_start(out=rwb_sb, in_=rwb[:])
        rrb_sb = consts.tile([128, NFC], F32)
        nc.sync.dma_start(out=rrb_sb, in_=rrb[:])
        ff1b_sb = consts.tile([128, NFF], F32)
        nc.sync.dma_start(out=ff1b_sb, in_=ff1b[:])

        def bcast(name, param):
            t = consts.tile([128, D], F32, tag=name)
            nc.sync.dma_start(out=t, in_=bass.AP(tensor=param, offset=0,
                                                 ap=[[0, 128], [1, D]]))
            return t

        ln1w_bc = bcast("ln1w_bc", ln1w)
        ln1b_bc = bcast("ln1b_bc", ln1b)
        ln2w_bc = bcast("ln2w_bc", ln2w)
        ln2b_bc = bcast("ln2b_bc", ln2b)

        # zero column 0 of both pad buffers (BD writes never touch col 0)
        for pad in pads:
            nc.sync.dma_start(
                out=bass.AP(tensor=pad, offset=0,
                            ap=[[K + 1, 128], [(K + 1) * 128, NT]]),
                in_=zrow)

        avT = ctx.enter_context(tc.tile_pool(name="avTp", bufs=1))
        avT_sb = avT.tile([128, NFC, K], BF16)

        attn_stack = ExitStack()
        attn = attn_stack.enter_context(tc.tile_pool(name="attn", bufs=1))
        q1T = attn.tile([128, NFC, K], BF16)
        q2T = attn.tile([128, NFC, K], BF16)
        kT = attn.tile([128, NFC, K], BF16)
        rT = attn.tile([128, NFC, K], BF16)
        v1 = attn.tile([128, NT, HC, DH], BF16)

        # ---------------- P0: LN1, transpose, projections ----------------
        with tc.tile_pool(name="p0", bufs=3) as p0, \
             tc.tile_pool(name="p0w", bufs=1) as p0w:
            # wT[p, t, dc, l] = w[t*128 + l, dc*128 + p]
            wT = p0w.tile([128, NT, ND, 128], BF16)

            for t in range(NT):
                x_t = p0.tile([128, D], F32, tag="x")
                nc.sync.dma_start(out=x_t, in_=xb[t * 128:(t + 1) * 128, :])
                mean, rstd = _ln_tile(nc, p0, x_t, eps_sb, stat_pool)
                wn = p0.tile([128, D], F32, tag="wn")
                nc.vector.tensor_scalar(out=wn, in0=x_t, scalar1=mean,
                                        scalar2=rstd,
                                        op0=mybir.AluOpType.subtract,
                                        op1=mybir.AluOpType.mult)
                wg = p0.tile([128, D], F32, tag="wg")
                nc.gpsimd.tensor_mul(out=wg, in0=wn, in1=ln1w_bc)
                wf = p0.tile([128, D], F32, tag="wf")
                nc.gpsimd.tensor_add(out=wf, in0=wg, in1=ln1b_bc)
                nc.sync.dma_start(out=w_dram[t * 128:(t + 1) * 128, :], in_=wf)
                wb = p0.tile([128, D], BF16, tag="wb")
                nc.scalar.copy(out=wb, in_=wf)
                nc.sync.dma_start_transpose(wT[:, t, :, :], wb)

            posT_sb = p0w.tile([128, NT, ND, 128], BF16)
            nc.scalar.dma_start(out=posT_sb, in_=posT[:])
            qw_sb = p0w.tile([128, ND, FC], BF16)
            nc.scalar.dma_start(out=qw_sb, in_=qw[:])
            kw_sb = p0w.tile([128, ND, FC], BF16)
            nc.scalar.dma_start(out=kw_sb, in_=kw[:])
            vw_sb = p0w.tile([128, ND, FC], BF16)
            nc.scalar.dma_start(out=vw_sb, in_=vw[:])
            rw_sb = p0w.tile([128, ND, FC], BF16)
            nc.scalar.dma_start(out=rw_sb, in_=rw[:])

            def projT(w_sb, dst, bias1=None, dst2=None, bias2=None,
                      rhs_sb=None):
                # dst[., fc, i] = (w_sb[:, :, fc-block].T @ w^T)[feat, tok]
                for fc in range(NFC):
                    ps = psS.tile([128, 1024], F32, tag="s", name=f"psp{fc}")
                    for dc in range(ND):
                        st = dict(start=(dc == 0), stop=(dc == ND - 1))
                        lh = w_sb[:, dc, fc * 128:(fc + 1) * 128]
                        r = rhs_sb if rhs_sb is not None else wT
                        nc.tensor.matmul(ps[:, 0:512], lh, r[:, 0:4, dc, :],
                                         **st)
                        nc.tensor.matmul(ps[:, 512:896], lh, r[:, 4:7, dc, :],
                                         **st)
                    if bias1 is not None:
                        nc.vector.tensor_scalar_add(
                            out=dst[:, fc, :], in0=ps[:, 0:K],
                            scalar1=bias1[:, fc:fc + 1])
                        nc.vector.tensor_scalar_add(
                            out=dst2[:, fc, :], in0=ps[:, 0:K],
                            scalar1=bias2[:, fc:fc + 1])
                    else:
                        nc.scalar.copy(out=dst[:, fc, :], in_=ps[:, 0:K])

            projT(qw_sb, q1T, bias1=rwb_sb, dst2=q2T, bias2=rrb_sb)
            projT(kw_sb, kT)
            projT(rw_sb, rT, rhs_sb=posT_sb)

            # v : [tok, feat] per head
            for t in range(NT):
                pv = psS.tile([128, 1024], F32, tag="s")
                for dc in range(ND):
                    nc.tensor.matmul(pv[:, 0:512], wT[:, t, dc, :],
                                     vw_sb[:, dc, :],
                                     start=(dc == 0), stop=(dc == ND - 1))
                nc.vector.tensor_copy(
                    out=v1[:, t, :, :],
                    in_=pv[:, 0:512].rearrange("p (h d) -> p h d", h=HC))

        if debug:
            for t in range(NT):
                tmp = stat_pool.tile([128, D], F32, tag="dbgw")
                nc.sync.dma_start(out=tmp, in_=w_dram[t * 128:(t + 1) * 128, :])
                nc.sync.dma_start(out=dbg_w[t * 128:(t + 1) * 128, :], in_=tmp)

        # ---------------- P1: per-head attention ----------------
        # S in [i, j] layout: both rel_shift DMAs are contiguous j-runs.
        # P row-blocks are transposed via one wide DMA-transpose per i-tile
        # into ptw[p, it, jt, l] = P[it*128 + l, jt*128 + p], enabling a
        # 14-matmul PV per head producing attn_vec^T directly.
        with tc.tile_pool(name="p1", bufs=3) as p1, \
             tc.tile_pool(name="ptw", bufs=2) as ptwp:
            for h in range(HC):
                pad = pads[h % 2]
                hp = (h % 2) * DH
                fc = h // 2
                ptw = ptwp.tile([128, NT, NT, 128], BF16, tag="ptw")

                def bd_step(it):
                    ps = psS.tile([128, 1024], F32, tag="s", name="psbd")
                    nc.tensor.matmul(
                        ps[:, 0:512],
                        q2T[hp:hp + DH, fc, it * 128:(it + 1) * 128],
                        rT[hp:hp + DH, fc, 0:512], start=True, stop=True)
                    nc.tensor.matmul(
                        ps[:, 512:896],
                        q2T[hp:hp + DH, fc, it * 128:(it + 1) * 128],
                        rT[hp:hp + DH, fc, 512:896], start=True, stop=True)
                    bd = p1.tile([128, K], BF16, tag="bd", name="bd")
                    nc.scalar.copy(out=bd, in_=ps[:, 0:K])
                    nc.scalar.dma_start(
                        out=bass.AP(tensor=pad,
                                    offset=(K + 1) * 128 * it + 1,
                                    ap=[[K + 1, 128], [1, K]]),
                        in_=bd)

                def s_step(it):
                    sbd = p1.tile([128, K], BF16, tag="sbd", name="sbd")
                    nc.sync.dma_start(
                        out=sbd,
                        in_=bass.AP(tensor=pad, offset=K + K * 128 * it,
                                    ap=[[K, 128], [1, K]]))
                    ps = psS.tile([128, 1024], F32, tag="s", name="psac")
                    nc.tensor.matmul(
                        ps[:, 0:512],
                        q1T[hp:hp + DH, fc, it * 128:(it + 1) * 128],
                        kT[hp:hp + DH, fc, 0:512], start=True, stop=True)
                    nc.tensor.matmul(
                        ps[:, 512:896],
                        q1T[hp:hp + DH, fc, it * 128:(it + 1) * 128],
                        kT[hp:hp + DH, fc, 512:896], start=True, stop=True)
                    nc.vector.tensor_add(out=ps[:, 0:K], in0=ps[:, 0:K],
                                         in1=sbd)
                    pexp = p1.tile([128, K], BF16, tag="pexp", name="pexp")
                    acc = stat_pool.tile([128, 1], F32, tag="acc", name="acc")
                    nc.scalar.activation(
                        out=pexp, in_=ps[:, 0:K],
                        func=mybir.ActivationFunctionType.Exp,
                        scale=float(SCALE), accum_out=acc)
                    rcp = stat_pool.tile([128, 1], F32, tag="rcp", name="rcp")
                    nc.vector.reciprocal(out=rcp, in_=acc)
                    nc.vector.tensor_scalar_mul(out=pexp, in0=pexp,
                                                scalar1=rcp)
                    nc.sync.dma_start_transpose(ptw[:, it, :, :], pexp)

                # interleave: s_step(it) only needs BD rows it and it+1
                for it in range(NT + 2):
                    if it < NT:
                        bd_step(it)
                    if it >= 2:
                        s_step(it - 2)
                if debug and h == 0:
                    total_rows = K * (K + 1) // 448
                    for r0 in range(0, total_rows, 128):
                        rows = min(128, total_rows - r0)
                        tmp = p1.tile([128, 512], BF16, tag="dbgp")
                        nc.sync.dma_start(
                            out=tmp[:rows, :448],
                            in_=bass.AP(tensor=pad, offset=r0 * 448,
                                        ap=[[448, rows], [1, 448]]))
                        nc.sync.dma_start(
                            out=bass.AP(tensor=dbg_pad, offset=r0 * 448,
                                        ap=[[448, rows], [1, 448]]),
                            in_=tmp[:rows, :448])

                pv0 = psS.tile([64, 512], F32, tag="pv", name="pv0", bufs=2)
                pv1 = psS.tile([64, 512], F32, tag="pv", name="pv1", bufs=2)
                for jt in range(NT):
                    st = dict(start=(jt == 0), stop=(jt == NT - 1))
                    nc.tensor.matmul(pv0, v1[:, jt, h, :],
                                     ptw[:, 0:4, jt, :], **st)
                    nc.tensor.matmul(pv1[:, 0:384], v1[:, jt, h, :],
                                     ptw[:, 4:7, jt, :], **st)
                nc.vector.tensor_copy(out=avT_sb[hp:hp + DH, fc, 0:512],
                                      in_=pv0)
                nc.vector.tensor_copy(out=avT_sb[hp:hp + DH, fc, 512:K],
                                      in_=pv1[:, 0:384])

        # ---------------- P2: o_proj, AllReduce, LN2, transpose ----------
        attn_stack.close()
        arT = ctx.enter_context(tc.tile_pool(name="arTp", bufs=1))
        arT_sb = arT.tile([128, NT, ND, 128], BF16)
        p23w = ctx.enter_context(tc.tile_pool(name="p23w", bufs=1))
        ow_sb = p23w.tile([128, NFC, D], BF16)
        nc.sync.dma_start(out=ow_sb, in_=ow[:])
        ff1w_sb = p23w.tile([128, ND, FFC], BF16)
        nc.scalar.dma_start(out=ff1w_sb, in_=ff1w[:])
        ff2w_sb = p23w.tile([128, NFF, D], BF16)
        nc.scalar.dma_start(out=ff2w_sb, in_=ff2w[:])
        hT = p23w.tile([128, NFF, K], BF16)

        with tc.tile_pool(name="p2", bufs=2) as p2:
            for it in range(NT):
                po = psS.tile([128, 1024], F32, tag="s", name="po")
                for fc in range(NFC):
                    st = dict(start=(fc == 0), stop=(fc == NFC - 1))
                    l = avT_sb[:, fc, it * 128:(it + 1) * 128]
                    nc.tensor.matmul(po[:, 0:512], l, ow_sb[:, fc, 0:512],
                                     **st)
                    nc.tensor.matmul(po[:, 512:1024], l, ow_sb[:, fc, 512:1024],
                                     **st)
                ao = p2.tile([128, D], F32, tag="ao")
                nc.vector.tensor_copy(out=ao, in_=po)
                nc.sync.dma_start(out=ao_in[it * 128:(it + 1) * 128, :], in_=ao)

            if sim_mode:
                nc.sync.dma_start(out=ao_out[:], in_=ao_in[:])
            else:
                nc.gpsimd.collective_compute(
                    "AllReduce", mybir.AluOpType.add,
                    replica_groups=[[0, 1], [2, 3], [4, 5], [6, 7]],
                    ins=[ao_in[:]], outs=[ao_out[:]])

            for it in range(NT):
                aor = p2.tile([128, D], F32, tag="aor")
                nc.sync.dma_start(out=aor,
                                  in_=ao_out[it * 128:(it + 1) * 128, :])
                if debug:
                    nc.sync.dma_start(out=dbg_ao[it * 128:(it + 1) * 128, :],
                                      in_=aor)
                wr = p2.tile([128, D], F32, tag="wr")
                nc.sync.dma_start(out=wr,
                                  in_=w_dram[it * 128:(it + 1) * 128, :])
                x2 = p2.tile([128, D], F32, tag="x2")
                nc.vector.tensor_add(out=x2, in0=aor, in1=wr)
                mean, rstd = _ln_tile(nc, p2, x2, eps_sb, stat_pool)
                nc.vector.tensor_scalar(out=x2, in0=x2, scalar1=mean,
                                        scalar2=rstd,
                                        op0=mybir.AluOpType.subtract,
                                        op1=mybir.AluOpType.mult)
                nc.gpsimd.tensor_mul(out=x2, in0=x2, in1=ln2w_bc)
                ars = p2.tile([128, D], BF16, tag="ars")
                nc.vector.tensor_add(out=ars, in0=x2, in1=ln2b_bc)
                if debug:
                    arf = p2.tile([128, D], F32, tag="arf")
                    nc.scalar.copy(out=arf, in_=ars)
                    nc.sync.dma_start(out=dbg_ares[it * 128:(it + 1) * 128, :],
                                      in_=arf)
                nc.sync.dma_start_transpose(arT_sb[:, it, :, :], ars)

        # ---------------- P3: FFN ----------------
        with tc.tile_pool(name="p3", bufs=3) as p3:
            for ffc in range(NFF):
                ph = psS.tile([128, 1024], F32, tag="s", name="ph")
                for dc in range(ND):
                    st = dict(start=(dc == 0), stop=(dc == ND - 1))
                    lh = ff1w_sb[:, dc, ffc * 128:(ffc + 1) * 128]
                    nc.tensor.matmul(ph[:, 0:512], lh, arT_sb[:, 0:4, dc, :],
                                     **st)
                    nc.tensor.matmul(ph[:, 512:896], lh, arT_sb[:, 4:7, dc, :],
                                     **st)
                nc.scalar.activation(
                    out=hT[:, ffc, :], in_=ph[:, 0:K],
                    func=mybir.ActivationFunctionType.Relu,
                    bias=ff1b_sb[:, ffc:ffc + 1], scale=1.0)
            for it in range(NT):
                po = psS.tile([128, 1024], F32, tag="s", name="pf")
                for ffc in range(NFF):
                    st = dict(start=(ffc == 0), stop=(ffc == NFF - 1))
                    l = hT[:, ffc, it * 128:(it + 1) * 128]
                    nc.tensor.matmul(po[:, 0:512], l, ff2w_sb[:, ffc, 0:512],
                                     **st)
                    nc.tensor.matmul(po[:, 512:1024], l,
                                     ff2w_sb[:, ffc, 512:1024], **st)
                ot = p3.tile([128, D], F32, tag="ot")
                nc.vector.tensor_copy(out=ot, in_=po)
                nc.sync.dma_start(out=out[it * 128:(it + 1) * 128, :], in_=ot)

    nc.finalize()
    return nc


def prep_inputs(inputs):
    """Full inputs -> list of 8 per-core input maps."""
    x_nat = np.concatenate([inputs["input_ids"], inputs["mem"],
                            inputs["c_mem"]], axis=0)  # [K,B,D] f32
    # posT[p, t, dc, l] = pos[t*128 + l, dc*128 + p]
    posn = inputs["positional_embedding"].astype(BF)  # [K, D]
    posT_t = np.ascontiguousarray(
        posn.reshape(NT, 128, ND, 128).transpose(3, 0, 2, 1))

    qkv = inputs["qkv_w"]
    maps = []
    for c in range(8):
        b, hh = c // 2, c % 2
        Fs = slice(hh * FC, (hh + 1) * FC)
        FFs = slice(hh * FFC, (hh + 1) * FFC)

        def wchunk(wmat):  # [D, FC] -> [128, ND, FC] bf16
            return np.ascontiguousarray(
                wmat.astype(BF).reshape(ND, 128, -1).transpose(1, 0, 2))

        m = {
            "xb": np.ascontiguousarray(x_nat[:, b, :], np.float32),
            "posT": posT_t,
            "qw": wchunk(qkv[:, 0 * H * DH:1 * H * DH][:, Fs]),
            "kw": wchunk(qkv[:, 1 * H * DH:2 * H * DH][:, Fs]),
            "vw": wchunk(qkv[:, 2 * H * DH:3 * H * DH][:, Fs]),
            "rw": wchunk(inputs["r_w"][:, Fs]),
            "ow": np.ascontiguousarray(
                inputs["o_w"][Fs, :].astype(BF)
                .reshape(NFC, 128, D).transpose(1, 0, 2)),
            "ff1w": wchunk(inputs["ff1_w"][:, FFs]),
            "ff2w": np.ascontiguousarray(
                inputs["ff2_w"][FFs, :].astype(BF)
                .reshape(NFF, 128, D).transpose(1, 0, 2)),
            "rwb": np.ascontiguousarray(
                inputs["r_w_bias"][hh * HC:(hh + 1) * HC].reshape(-1)
                .reshape(NFC, 128).T.astype(np.float32)),
            "rrb": np.ascontiguousarray(
                inputs["r_r_bias"][hh * HC:(hh + 1) * HC].reshape(-1)
                .reshape(NFC, 128).T.astype(np.float32)),
            "ff1b": np.ascontiguousarray(
                inputs["ff1_b"][FFs].reshape(NFF, 128).T.astype(np.float32)),
            "ln1w": np.asarray(inputs["ln1_w"], np.float32),
            "ln1b": np.asarray(inputs["ln1_b"], np.float32),
            "ln2w": np.asarray(inputs["ln2_w"], np.float32),
            "ln2b": np.asarray(inputs["ln2_b"], np.float32),
        }
        maps.append(m)
    return maps


class PjrtRunner:
    """Persistent jitted SPMD executor for a prebuilt Bass module.

    Mirrors bass2jax.run_bass_via_pjrt but keeps the jitted callable so
    repeated invocations (for steady-state timing) skip retrace/recompile.
    Donation is disabled; every output element is written by the kernel.
    """

    def __init__(self, nc, n_cores=8):
        import jax
        from jax.sharding import Mesh, PartitionSpec
        from jax.experimental.shard_map import shard_map
        from concourse import mybir as _mybir
        from concourse.bass2jax import (_bass_exec_p, install_neuronx_cc_hook,
                                        partition_id_tensor)
        install_neuronx_cc_hook()
        self.jax = jax
        self.n_cores = n_cores
        in_names, out_names, out_avals = [], [], []
        partition_name = (nc.partition_id_tensor.name
                          if nc.partition_id_tensor else None)
        for alloc in nc.m.functions[0].allocations:
            if not isinstance(alloc, _mybir.MemoryLocationSet):
                continue
            name = alloc.memorylocations[0].name
            if alloc.kind == "ExternalInput":
                if name != partition_name:
                    in_names.append(name)
            elif alloc.kind == "ExternalOutput":
                out_names.append(name)
                out_avals.append(jax.core.ShapedArray(
                    tuple(alloc.tensor_shape), _mybir.dt.np(alloc.dtype)))
        self.in_names, self.out_names, self.out_avals = \
            in_names, out_names, out_avals

        def _body(*args):
            operands = list(args)
            if partition_name is not None:
                operands.append(partition_id_tensor())
            all_in = in_names + out_names
            if partition_name is not None:
                all_in = all_in + [partition_name]
            return tuple(_bass_exec_p.bind(
                *operands,
                out_avals=tuple(out_avals),
                in_names=tuple(all_in),
                out_names=tuple(out_names),
                lowering_input_output_aliases=(),
                sim_require_finite=True,
                sim_require_nnan=True,
                nc=nc,
            ))

        devices = jax.devices()[:n_cores]
        self.mesh = Mesh(np.asarray(devices), ("core",))
        nin = len(in_names) + len(out_names)
        self.fn = jax.jit(shard_map(
            _body, mesh=self.mesh,
            in_specs=(PartitionSpec("core"),) * nin,
            out_specs=(PartitionSpec("core"),) * len(out_names),
            check_rep=False))

    def pack(self, maps):
        """Per-core input maps -> list of concatenated device arrays."""
        arrs = [self.jax.device_put(
                    np.concatenate([np.asarray(maps[c][n])
                                    for c in range(self.n_cores)], axis=0))
                for n in self.in_names]
        arrs += [self.jax.device_put(
                    np.zeros((self.n_cores * a.shape[0], *a.shape[1:]),
                             a.dtype))
                 for a in self.out_avals]
        return arrs

    def __call__(self, packed):
        return self.fn(*packed)

    def unpack(self, outs):
        res = []
        for c in range(self.n_cores):
            res.append({
                n: np.asarray(outs[i]).reshape(
                    self.n_cores, *self.out_avals[i].shape)[c]
                for i, n in enumerate(self.out_names)})
        return res


def get_runner(debug=False, sim_mode=False):
    key = (bool(debug), bool(sim_mode))
    if key not in _CACHED:
        nc = build(debug=debug, sim_mode=sim_mode)
        _CACHED[key] = PjrtRunner(nc, 8)
    return _CACHED[key]


def _assemble(inputs, results):
    ff2b = np.asarray(inputs["ff2_b"], np.float32)
    out = np.zeros((K, B, D), np.float32)
    for b in range(B):
        out[:, b, :] = (np.asarray(results[2 * b]["out"], np.float32)
                        + np.asarray(results[2 * b + 1]["out"], np.float32)
                        + ff2b[None, :])
    return out


def run(inputs, trace=False, debug=False):
    runner = get_runner(debug=debug)
    maps = prep_inputs(inputs)
    packed = runner.pack(maps)
    outs = runner(packed)
    results = runner.unpack(outs)

    class R:
        pass
    res = R()
    res.results = results
    res.exec_time_ns = None
    return _assemble(inputs, results), res


def kernel(**inputs):
    inputs = {k: np.asarray(v) for k, v in inputs.items()}
    out, _ = run(inputs, trace=False, debug=False)
    return out
